# revision 8
# baseline (speedup 1.0000x reference)
"""KGAN-style 2-hop knowledge-graph attention kernel for 8 Trainium2 NeuronCores.

Strategy (data-parallel over batch; tables replicated):
  - Each core handles BL=32 batches. Groups g=(b_local, r) -> 512 per hop,
    processed as tiles of 128 groups (8 tiles total = 2 hops x 4).
  - Entity rows for h/t and the item embedding v are gathered with
    indirect_dma_start (128x32 rows / instruction, 256B rows).
  - Relation modulation: host builds a 256x128 f32 "relation pair" table
    (row k1*16+k2 = [rel[k1], rel[k2]]); the kernel gathers memory-PAIRS
    with combined indices -> 512B descriptors, no small-descriptor penalty,
    and no per-element on-chip select is needed.
  - scores = reduce_d(h * relexp * v_bcast); softmax over m on ACT with
    fused exp+sum; out = reduce_m(t * p) / sum.
"""

import numpy as np

N_ENT = 500001
B = 256
R = 16
D = 64
M = 32
HOPS = 2
NCORES = 8
BL = B // NCORES          # 32 local batches per core
G = BL * R                # 512 groups (b, r) per hop per core
TPH = G // 128            # 4 tiles of 128 groups per hop
TILES = HOPS * TPH        # 8 tiles per core
MQ = M // 4               # 8 memory-quads per group

_NC = None


def _build_program():
    import concourse.bacc as bacc
    import concourse.bass as bass
    import concourse.tile as tile
    from concourse import mybir

    dt = mybir.dt
    f32 = dt.float32
    i32 = dt.int32
    Alu = mybir.AluOpType
    Axis = mybir.AxisListType

    nc = bacc.Bacc("TRN2", debug=False, num_devices=NCORES)

    ent = nc.dram_tensor("entity", (N_ENT, D), f32, kind="ExternalInput").ap()
    relp = nc.dram_tensor("relpair", (R**4, 4 * D), f32, kind="ExternalInput").ap()
    mh = nc.dram_tensor("mh", (128, TILES * M), i32, kind="ExternalInput").ap()
    mt = nc.dram_tensor("mt", (128, TILES * M), i32, kind="ExternalInput").ap()
    cp = nc.dram_tensor("cp", (128, TILES * MQ), i32, kind="ExternalInput").ap()
    vi = nc.dram_tensor("vi", (128, TPH), i32, kind="ExternalInput").ap()
    out = nc.dram_tensor("out", (TILES, 128, D), f32, kind="ExternalOutput").ap()

    with tile.TileContext(nc) as tc:
        with (
            tc.tile_pool(name="idx", bufs=1) as idxp,
            tc.tile_pool(name="gat", bufs=3) as gat,
            tc.tile_pool(name="wrk", bufs=2) as wrk,
            tc.tile_pool(name="sml", bufs=2) as sml,
        ):
            mh_sb = idxp.tile([128, TILES * M], i32)
            mt_sb = idxp.tile([128, TILES * M], i32)
            cp_sb = idxp.tile([128, TILES * MQ], i32)
            vi_sb = idxp.tile([128, TPH], i32)
            nc.sync.dma_start(out=mh_sb, in_=mh)
            nc.sync.dma_start(out=mt_sb, in_=mt)
            nc.sync.dma_start(out=cp_sb, in_=cp)
            nc.sync.dma_start(out=vi_sb, in_=vi)

            # v embeddings for all 4 group-tiles (hop independent): [128, 4*64]
            # HW indirect-DMA semantics (probe-verified): ONE index per
            # partition per instruction; dest = [128, rowlen] column slice.
            v_sb = idxp.tile([128, TPH * D], f32)
            for j in range(TPH):
                nc.gpsimd.indirect_dma_start(
                    out=v_sb[:, j * D : (j + 1) * D],
                    out_offset=None,
                    in_=ent,
                    in_offset=bass.IndirectOffsetOnAxis(
                        ap=vi_sb[:, j : j + 1], axis=0
                    ),
                )

            for t in range(TILES):
                j = t % TPH  # group-tile index within hop

                h_t = gat.tile([128, M * D], f32, tag="h")
                for m in range(M):
                    nc.gpsimd.indirect_dma_start(
                        out=h_t[:, m * D : (m + 1) * D],
                        out_offset=None,
                        in_=ent,
                        in_offset=bass.IndirectOffsetOnAxis(
                            ap=mh_sb[:, t * M + m : t * M + m + 1], axis=0
                        ),
                    )
                r_t = gat.tile([128, M * D], f32, tag="r")
                for q in range(MQ):
                    nc.gpsimd.indirect_dma_start(
                        out=r_t[:, q * 4 * D : (q + 1) * 4 * D],
                        out_offset=None,
                        in_=relp,
                        in_offset=bass.IndirectOffsetOnAxis(
                            ap=cp_sb[:, t * MQ + q : t * MQ + q + 1], axis=0
                        ),
                    )
                t_t = gat.tile([128, M * D], f32, tag="t")
                for m in range(M):
                    nc.gpsimd.indirect_dma_start(
                        out=t_t[:, m * D : (m + 1) * D],
                        out_offset=None,
                        in_=ent,
                        in_offset=bass.IndirectOffsetOnAxis(
                            ap=mt_sb[:, t * M + m : t * M + m + 1], axis=0
                        ),
                    )

                # hr = h * relexp ; hrv = hr * v (v broadcast over m)
                hr = wrk.tile([128, M * D], f32, tag="hr")
                nc.vector.tensor_tensor(out=hr, in0=h_t, in1=r_t, op=Alu.mult)
                hrv = wrk.tile([128, M * D], f32, tag="hrv")
                v_b = (
                    v_sb[:, j * D : (j + 1) * D]
                    .rearrange("p (o d) -> p o d", o=1)
                    .to_broadcast([128, M, D])
                )
                nc.vector.tensor_tensor(out=hrv, in0=hr, in1=v_b, op=Alu.mult)

                # scores[g, m] = sum_d hrv
                scores = sml.tile([128, M], f32, tag="sc")
                nc.vector.tensor_reduce(
                    out=scores,
                    in_=hrv.rearrange("p (m d) -> p m d", d=D),
                    axis=Axis.X,
                    op=Alu.add,
                )

                # softmax over m (fused exp+sum on ACT)
                nmax = sml.tile([128, 1], f32, tag="nm")
                nc.vector.tensor_reduce(
                    out=nmax, in_=scores, axis=Axis.X, op=Alu.max, negate=True
                )
                sexp = sml.tile([128, M], f32, tag="se")
                ssum = sml.tile([128, 1], f32, tag="ss")
                nc.scalar.activation(
                    out=sexp,
                    in_=scores,
                    func=mybir.ActivationFunctionType.Exp,
                    bias=nmax,
                    scale=1.0,
                    accum_out=ssum,
                )
                srec = sml.tile([128, 1], f32, tag="sr")
                nc.vector.reciprocal(out=srec, in_=ssum)

                # out[g, d] = (sum_m t * exp) / sum
                tp = wrk.tile([128, M * D], f32, tag="tp")
                se_b = (
                    sexp.rearrange("p (m o) -> p m o", o=1)
                    .to_broadcast([128, M, D])
                )
                nc.vector.tensor_tensor(out=tp, in0=t_t, in1=se_b, op=Alu.mult)
                outr = sml.tile([128, D], f32, tag="or")
                nc.vector.tensor_reduce(
                    out=outr,
                    in_=tp.rearrange("p (m d) -> p d m", d=D),
                    axis=Axis.X,
                    op=Alu.add,
                )
                out_t = sml.tile([128, D], f32, tag="ot")
                nc.vector.tensor_scalar_mul(out=out_t, in0=outr, scalar1=srec)
                nc.sync.dma_start(out=out[t], in_=out_t)

    nc.compile()
    return nc


def _get_nc():
    global _NC
    if _NC is None:
        _NC = _build_program()
    return _NC


def _prep_shared(entity_emb, relation_emb):
    ent = np.ascontiguousarray(np.asarray(entity_emb, dtype=np.float32))
    rel = np.ascontiguousarray(np.asarray(relation_emb, dtype=np.float32))
    # quad table: row ((k0*R+k1)*R+k2)*R+k3 = [rel[k0]|rel[k1]|rel[k2]|rel[k3]]
    relpair = np.ascontiguousarray(
        np.concatenate(
            [
                np.repeat(rel, R**3, axis=0),
                np.tile(np.repeat(rel, R**2, axis=0), (R, 1)),
                np.tile(np.repeat(rel, R, axis=0), (R**2, 1)),
                np.tile(rel, (R**3, 1)),
            ],
            axis=1,
        )
    )
    return ent, relpair


def _swizzle_idx(a):
    """[HOPS, G, K] -> [128, TILES*K] with tile t=(h*TPH+j) rows g=j*128+p."""
    hops, g, k = a.shape
    assert hops == HOPS and g == G
    return np.ascontiguousarray(
        a.reshape(HOPS, TPH, 128, k).transpose(2, 0, 1, 3).reshape(128, TILES * k)
    )


def _prep_core(items32, mh_c, mr_c, mt_c):
    """Per-core index prep. mh_c/mr_c/mt_c: [HOPS, G, M] int32, items32: [BL]."""
    mh_host = _swizzle_idx(mh_c)
    mt_host = _swizzle_idx(mt_c)
    cp_c = (
        ((mr_c[..., 0::4] * R + mr_c[..., 1::4]) * R + mr_c[..., 2::4]) * R
        + mr_c[..., 3::4]
    )  # [HOPS, G, MQ]
    cp_host = _swizzle_idx(cp_c)
    ve = np.repeat(items32, R)  # [G]
    vi_host = np.ascontiguousarray(ve.reshape(TPH, 128).T)  # [128, TPH]
    return mh_host, mt_host, cp_host, vi_host


def make_in_maps(**inputs):
    ent, relpair = _prep_shared(inputs["entity_emb"], inputs["relation_emb"])
    items32 = np.asarray(inputs["items"], dtype=np.int32)
    mh_all = np.asarray(inputs["memories_h"], dtype=np.int32)
    mr_all = np.asarray(inputs["memories_r"], dtype=np.int32)
    mt_all = np.asarray(inputs["memories_t"], dtype=np.int32)

    in_maps = []
    for c in range(NCORES):
        bsl = slice(c * BL, (c + 1) * BL)
        mh_c = mh_all[:, bsl].reshape(HOPS, G, M)
        mr_c = mr_all[:, bsl].reshape(HOPS, G, M)
        mt_c = mt_all[:, bsl].reshape(HOPS, G, M)
        mh_host, mt_host, cp_host, vi_host = _prep_core(
            items32[bsl], mh_c, mr_c, mt_c
        )
        in_maps.append(
            {
                "entity": ent,
                "relpair": relpair,
                "mh": mh_host,
                "mt": mt_host,
                "cp": cp_host,
                "vi": vi_host,
            }
        )
    return in_maps


def assemble_output(per_core_outs):
    full = np.zeros((HOPS, B, R, D), np.float32)
    for c in range(NCORES):
        o = np.asarray(per_core_outs[c]).reshape(HOPS, TPH * 128, D)
        full[:, c * BL : (c + 1) * BL] = o.reshape(HOPS, BL, R, D)
    return full


def run_on_cores(in_maps, trace=False):
    from concourse.bass_utils import run_bass_kernel_spmd

    nc = _get_nc()
    return run_bass_kernel_spmd(
        nc, in_maps, core_ids=list(range(NCORES)), trace=trace
    )


def kernel(**inputs):
    in_maps = make_in_maps(**inputs)
    res = run_on_cores(in_maps, trace=False)
    return assemble_output([r["out"] for r in res.results])
